# revision 31
# baseline (speedup 1.0000x reference)
"""GNN message-passing (segment-softmax attention aggregation) on 8 TRN2 cores.

V2 strategy (node-sharded, 4-pass chunked gather, pipelined):
- Nodes sorted by total degree -> canonical positions (padded to NPOS).
  Canonical group g = pos//128 is owned by core g%8. NLOC = NPOS/8 per core.
- KV table is built per dst CHUNK (uneven chunks [8192, 31744, 31744, 30720]
  so pass 0's table is ready almost immediately); chunk rows <= 32767 so
  dma_gather's int16 indices can address them.
- Phase B pass q: core's nodes re-sorted by per-pass degree, grouped 128 at
  a time, padded to batch-common degree d with G*d <= W_CAP. One dma_gather
  call per batch (the GPSIMD descriptor generation at ~7ns/desc is the
  kernel's bottleneck, so calls are as large as SBUF allows). DVE computes
  scores q.k/8, exp, per-node partials (num = sum e*V, den = sum e).
- Passes 0-2 write raw partials to HBM (parts01 [2*NLOC,128] / parts2).
  Pass 3 fuses the combine: per batch it additionally gathers the pass-0/1/2
  partial rows for its nodes (pass-3 order), sums, divides, and writes the
  output rows directly (in pass-3 order; the host unshard remaps).
- Phase A work for chunk q+1 (KV matmuls) and the pass-q+1 Q tile are
  emitted interleaved with pass-q batches so PE/DMA hide under GPSIMD.

Softmax max-subtraction is skipped: scores are ~N(0, 0.33^2) here, exp is
safe in fp32 and softmax is shift-invariant, so results match the reference
to fp32 rounding.
"""

import math
import sys

import ml_dtypes

import numpy as np

for _p in ("/opt/trn_rl_repo", "/root/.axon_site/_ro/trn_rl_repo"):
    if _p not in sys.path:
        sys.path.append(_p)

P = 128
NC = 8
W_CAP = 48       # max slot columns per gather/DVE batch
G_CAP = 32       # max groups per batch
CG_CAP = 8       # max groups per fused-combine batch (pass 3)
NEG = -1.0e30    # additive mask for padded slots


def _cfg_from_shapes(N, D, H):
    NPOS = ((N + 1023) // 1024) * 1024          # multiple of 128*8
    NG = NPOS // P                              # canonical groups
    NK = NG // NC                               # groups per core
    NLOC = NK * P                               # nodes per core
    # uneven dst chunks: small first chunk so pass 0 starts early.
    if NPOS <= 16384:
        c0 = c1 = NPOS // 4
    else:
        c0 = min(12288, NPOS - 3 * 1024)
        c1 = min(((NPOS - c0 + 1) // 2 + 1023) // 1024 * 1024, 30720)
    bounds = [0, c0]
    while bounds[-1] < NPOS:
        bounds.append(min(bounds[-1] + c1, NPOS))
    assert len(bounds) == 5, bounds
    chunks = [bounds[i + 1] - bounds[i] for i in range(4)]
    assert all(c <= 32767 for c in chunks), chunks
    assert NLOC <= 32767, NLOC
    return dict(N=N, D=D, H=H, NPOS=NPOS, NG=NG, NK=NK, NLOC=NLOC,
                BOUNDS=bounds, CHUNKS=chunks)


def _wrap_idx(logical):
    """dma_gather index layout: logical i lives at [i%16, i//16], replicated
    across the 8 GPSIMD cores (128 partitions)."""
    num = logical.shape[0]
    assert num % 16 == 0
    w16 = logical.astype(np.int16).reshape(num // 16, 16).T  # [16, num/16]
    return np.tile(w16, (8, 1))                              # [128, num/16]


def _prep(cfg, X, Wq, Wk, Wv, edge_index):
    N, D, H = cfg["N"], cfg["D"], cfg["H"]
    NPOS, NK, NLOC = cfg["NPOS"], cfg["NK"], cfg["NLOC"]
    BOUNDS = np.asarray(cfg["BOUNDS"], np.int64)
    NDUM = NPOS - N

    src = np.asarray(edge_index[0], dtype=np.int64)
    dst = np.asarray(edge_index[1], dtype=np.int64)
    E = src.shape[0]

    deg = np.bincount(src, minlength=N)
    order = np.argsort(deg, kind="stable")          # real nodes, degree asc
    pos_of = np.empty(N, np.int64)
    pos_of[order] = NDUM + np.arange(N)             # canonical position

    def loc_of(pos):
        return (pos // (P * NC)) * P + pos % P      # canonical-local row

    spos = pos_of[src]
    dpos = pos_of[dst]
    ecore = (spos // P) % NC
    eq = np.searchsorted(BOUNDS, dpos, side="right") - 1   # pass (dst chunk)
    sloc = loc_of(spos)

    # per (core, pass, node-loc) degree
    key_cqn = (ecore * 4 + eq) * NLOC + sloc
    deg_cqn = np.bincount(key_cqn, minlength=NC * 4 * NLOC).reshape(NC, 4, NLOC)

    # pass orderings per (core, pass)
    pq_order = np.empty((NC, 4, NLOC), np.int64)
    pq_pos = np.empty((NC, 4, NLOC), np.int64)
    for c in range(NC):
        for q in range(4):
            o = np.argsort(deg_cqn[c, q], kind="stable")
            pq_order[c, q] = o
            pq_pos[c, q, o] = np.arange(NLOC)

    # common d-schedule per pass: d_q[k] = max over cores of group max degree
    d_sched = np.empty((4, NK), np.int64)
    for q in range(4):
        for k in range(NK):
            mx = 0
            for c in range(NC):
                sd = deg_cqn[c, q][pq_order[c, q, (k + 1) * P - 1]]
                mx = max(mx, int(sd))
            d_sched[q, k] = max(mx, 1)

    # batch schedules: consecutive groups, common padded degree d, G*d <=
    # W_CAP (unless d alone exceeds it), G <= G_CAP.  Pass 3 additionally
    # caps G at CG_CAP (fused combine gathers 2*G*128 partial rows).
    sched = []          # sched[q] = list of (k0, G, d, col0)
    totw = []
    for q in range(4):
        gcap = G_CAP if q < 3 else CG_CAP
        batches = []
        col0 = 0
        k = 0
        while k < NK:
            g = 1
            while (
                k + g < NK
                and g < gcap
                and d_sched[q, k + g] == d_sched[q, k]
                and (g + 1) * d_sched[q, k] <= W_CAP
            ):
                g += 1
            d = int(d_sched[q, k + g - 1])
            batches.append((k, g, d, col0))
            col0 += g * d
            k += g
        sched.append(batches)
        totw.append(col0)

    # column base per (q, k)
    colbase = np.zeros((4, NK), np.int64)
    for q in range(4):
        for (k0, g, d, col0) in sched[q]:
            for kk in range(k0, k0 + g):
                colbase[q, kk] = col0 + (kk - k0) * d

    # per-edge slot assignment
    eorder = np.argsort(key_cqn, kind="stable")
    counts = np.bincount(key_cqn, minlength=NC * 4 * NLOC)
    starts = np.concatenate(([0], np.cumsum(counts)))[:-1]
    j_of = np.arange(E) - starts[key_cqn[eorder]]

    se_sloc = sloc[eorder]
    se_core = ecore[eorder]
    se_q = eq[eorder]
    se_dpos = dpos[eorder]
    pq_e = pq_pos[se_core, se_q, se_sloc]
    k_e = pq_e // P
    p_e = pq_e % P
    col_e = colbase[se_q, k_e] + j_of

    # assemble per (core, pass) kv index grids + masks
    kvidx2d = [[np.zeros((P, totw[q]), np.int64) for q in range(4)] for _ in range(NC)]
    gmask2d = [[np.full((P, totw[q]), NEG, np.float32) for q in range(4)] for _ in range(NC)]
    cq_key = se_core * 4 + se_q
    cq_counts = np.bincount(cq_key, minlength=NC * 4)
    cq_starts = np.concatenate(([0], np.cumsum(cq_counts)))
    for c in range(NC):
        for q in range(4):
            a, b = cq_starts[c * 4 + q], cq_starts[c * 4 + q + 1]
            pp = p_e[a:b]
            cc = col_e[a:b]
            kvidx2d[c][q][pp, cc] = se_dpos[a:b] - BOUNDS[q]
            gmask2d[c][q][pp, cc] = 0.0

    # wrapped kv indices (j-major per batch), concatenated over batches/passes
    kvw_cols = []       # per (q, batch): wrapped col offset in the concat
    kvw_parts = [[] for _ in range(NC)]
    off = 0
    for q in range(4):
        qcols = []
        for (k0, g, d, col0) in sched[q]:
            w = g * d
            qcols.append(off)
            off += (P * w) // 16
            for c in range(NC):
                block = kvidx2d[c][q][:, col0:col0 + w]      # [128, w]
                logical = block.T.ravel()                    # i = col*128 + p
                kvw_parts[c].append(_wrap_idx(logical))
        kvw_cols.append(qcols)
    KVIW = off
    kvidx_w = [np.concatenate(kvw_parts[c], axis=1) for c in range(NC)]

    # gmask concat (per pass 2D layout back-to-back)
    gm_off = np.concatenate(([0], np.cumsum(totw)))[:4]
    gmask = [np.concatenate([gmask2d[c][q] for q in range(4)], axis=1) for c in range(NC)]

    # fused-combine gather indices for pass-3 batches:
    #   parts01 rows: q*NLOC + pq_pos[c, q, node] for q in (0, 1)
    #   parts2  rows: pq_pos[c, 2, node]
    # node at pass-3 position r3 = (k0+g)*P + p is pq_order[c, 3, r3];
    # call layout: i = (q*G + g)*128 + p.
    c01_parts = [[] for _ in range(NC)]
    c2_parts = [[] for _ in range(NC)]
    c01_cols = []       # per batch: wrapped col offset
    c2_cols = []
    o01 = o2 = 0
    for (k0, g, d, col0) in sched[3]:
        c01_cols.append(o01)
        c2_cols.append(o2)
        o01 += (2 * g * P) // 16
        o2 += (g * P) // 16
        for c in range(NC):
            nodes = pq_order[c, 3, k0 * P:(k0 + g) * P]      # [(g p)] order
            l0 = 0 * NLOC + pq_pos[c, 0, nodes]
            l1 = 1 * NLOC + pq_pos[c, 1, nodes]
            c01_parts[c].append(_wrap_idx(np.concatenate([l0, l1])))
            c2_parts[c].append(_wrap_idx(pq_pos[c, 2, nodes]))
    C01W, C2W = o01, o2
    c01idx = [np.concatenate(c01_parts[c], axis=1) for c in range(NC)]
    c2idx = [np.concatenate(c2_parts[c], axis=1) for c in range(NC)]

    # X tables (canonical order, transposed), weights
    BF = ml_dtypes.bfloat16
    Xp = np.zeros((NPOS, D), np.float32)
    Xp[NDUM + np.arange(N)] = np.asarray(X, np.float32)[order]
    xt = np.ascontiguousarray(Xp.T.astype(BF))          # [D, NPOS] bf16
    # own-node X.T in canonical-local order (Q computed once on device into
    # qt; per-pass Q tiles come from dma_gather of qt rows in pass order)
    xtqc = []
    qgidx = []
    kk = np.arange(NLOC)
    for c in range(NC):
        gpos = ((kk // P) * NC + c) * P + kk % P        # canonical positions
        Xloc = Xp[gpos]                                 # [NLOC, D] canonical-local
        xtqc.append(np.ascontiguousarray(Xloc.T.astype(BF)))
        qgidx.append(np.concatenate(
            [_wrap_idx(pq_order[c, q]) for q in range(4)], axis=1))
    w = np.concatenate(
        [np.asarray(Wk, np.float32), np.asarray(Wv, np.float32), np.asarray(Wq, np.float32)],
        axis=1,
    ).astype(BF)                                         # [D, 3H] bf16

    assert int(d_sched.max()) <= W_CAP, int(d_sched.max())
    assert max(totw) <= 576, totw

    meta = dict(sched=sched, kvw_cols=kvw_cols, gm_off=gm_off.tolist(),
                KVIW=KVIW, TOTW=int(sum(totw)), totw=[int(t) for t in totw],
                C01W=C01W, C2W=C2W, c01_cols=c01_cols, c2_cols=c2_cols,
                QGW=NLOC // 16)
    in_maps = []
    for c in range(NC):
        m = {
            "xt": xt, "w": w,
            "kvidx": np.ascontiguousarray(kvidx_w[c]),
            "gmask": np.ascontiguousarray(gmask[c]),
            "c01idx": np.ascontiguousarray(c01idx[c]),
            "c2idx": np.ascontiguousarray(c2idx[c]),
            "xtqc": xtqc[c],
            "qgidx": np.ascontiguousarray(qgidx[c]),
        }
        in_maps.append(m)

    post = dict(order=order, NDUM=NDUM, pq_order3=pq_order[:, 3].copy())
    return meta, in_maps, post


def _build_program(cfg, meta):
    import concourse.bass as bass
    import concourse.tile as tile
    from concourse import bacc, mybir

    f32 = mybir.dt.float32
    bf16 = mybir.dt.bfloat16
    i16 = mybir.dt.int16
    AF = mybir.ActivationFunctionType
    OP = mybir.AluOpType
    AX = mybir.AxisListType

    D, H = cfg["D"], cfg["H"]
    NPOS, NK, NLOC = cfg["NPOS"], cfg["NK"], cfg["NLOC"]
    BOUNDS, CHUNKS = cfg["BOUNDS"], cfg["CHUNKS"]
    H2 = 2 * H
    DC = D // P                      # contraction chunks (2 for D=256)
    sched = meta["sched"]
    kvw_cols = meta["kvw_cols"]
    gm_off = meta["gm_off"]
    totw = meta["totw"]
    dk_scale = 1.0 / math.sqrt(H)

    NQ = 4
    nc = bacc.Bacc(num_swdge_queues=NQ)
    xt = nc.declare_dram_parameter("xt", [D, NPOS], bf16, isOutput=False)
    xtqc = nc.declare_dram_parameter("xtqc", [D, NLOC], bf16, isOutput=False)
    qgidx = nc.declare_dram_parameter("qgidx", [P, 4 * meta["QGW"]], i16,
                                      isOutput=False)
    w = nc.declare_dram_parameter("w", [D, 3 * H], bf16, isOutput=False)
    kvidx = nc.declare_dram_parameter("kvidx", [P, meta["KVIW"]], i16, isOutput=False)
    gmask = nc.declare_dram_parameter("gmask", [P, meta["TOTW"]], f32, isOutput=False)
    c01idx = nc.declare_dram_parameter("c01idx", [P, meta["C01W"]], i16, isOutput=False)
    c2idx = nc.declare_dram_parameter("c2idx", [P, meta["C2W"]], i16, isOutput=False)
    out = nc.declare_dram_parameter("out", [NLOC, H], f32, isOutput=True)

    kvts = [nc.dram_tensor(f"kvt{q}", [CHUNKS[q], H2], bf16) for q in range(4)]
    qt = nc.dram_tensor("qt", [NLOC, H], f32)
    parts01 = nc.dram_tensor("parts01", [2 * NLOC, H2], f32)
    parts2 = nc.dram_tensor("parts2", [NLOC, H2], f32)

    from contextlib import ExitStack
    with tile.TileContext(nc) as tc, ExitStack() as ctx0:
        cpool = ctx0.enter_context(tc.tile_pool(name="const", bufs=1))
        w_sb = cpool.tile([P, DC, 3 * H], bf16)
        nc.sync.dma_start(w_sb[:], w[:].rearrange("(c p) m -> p c m", p=P))

        pq = ctx0.enter_context(tc.tile_pool(name="pq", bufs=1))
        pqg = ctx0.enter_context(tc.tile_pool(name="pqg", bufs=1))
        pq_ps = ctx0.enter_context(tc.tile_pool(name="pq_ps", bufs=2, space="PSUM"))
        pq_x = ctx0.enter_context(tc.tile_pool(name="pq_x", bufs=2))
        pa = ctx0.enter_context(tc.tile_pool(name="pa", bufs=2))
        pa_ps = ctx0.enter_context(tc.tile_pool(name="pa_ps", bufs=2, space="PSUM"))
        pa_st = ctx0.enter_context(tc.tile_pool(name="pa_st", bufs=2))
        pidx = ctx0.enter_context(tc.tile_pool(name="pidx", bufs=2))
        pg = ctx0.enter_context(tc.tile_pool(name="pg", bufs=3))
        pcmb = ctx0.enter_context(tc.tile_pool(name="pcmb", bufs=2))
        pbs = ctx0.enter_context(tc.tile_pool(name="pbs", bufs=2))

        def emit_qphase():
            """Compute qt[NLOC, H] f32 once from canonical-local own X."""
            QB = 8
            b0 = 0
            while b0 < NK:
                qb = min(QB, NK - b0)
                m0 = b0 * P
                xqb = pq_x.tile([P, QB, DC, P], bf16, tag="xqb")
                for c in range(DC):
                    nc.sync.dma_start(
                        xqb[:, :qb, c, :],
                        xtqc[c * P:(c + 1) * P, m0:m0 + qb * P]
                        .rearrange("p (t n) -> p t n", n=P))
                psq = pq_ps.tile([P, QB * H], f32, tag="psQ")
                psqv = psq[:].rearrange("p (t e) -> p t e", e=H)
                for t in range(qb):
                    for c in range(DC):
                        nc.tensor.matmul(
                            psqv[:, t, :], lhsT=xqb[:, t, c, :],
                            rhs=w_sb[:, c, H2:3 * H],
                            start=(c == 0), stop=(c == DC - 1))
                qst = pq_x.tile([P, QB * H], f32, tag="qst")
                nc.scalar.activation(qst[:, :qb * H], psq[:, :qb * H], AF.Copy)
                nc.sync.dma_start(
                    qt[m0:m0 + qb * P, :].rearrange("(t p) h -> p t h", p=P),
                    qst[:, :qb * H].rearrange("p (t h) -> p t h", h=H))
                b0 += qb

        def emit_qgather(q):
            """Gather qt rows in pass-q order -> bf16 qtile [P, NK*H]."""
            qg = pqg.tile([P, NK * H], f32, tag="qg")
            qgv = qg[:].rearrange("p (w e) -> p w e", e=H)
            QGW = meta["QGW"]
            c0 = 0
            while c0 < NLOC:
                cw = min(1024, NLOC - c0)
                qn = qcycle[0]
                qcycle[0] = (qn + 1) % NQ
                nc.gpsimd.dma_gather(
                    out_ap=qgv[:, c0 // P:(c0 + cw) // P, :],
                    in_ap=qt[:],
                    idxs_ap=qgidx_sb[:, q * QGW + c0 // 16:
                                     q * QGW + (c0 + cw) // 16],
                    num_idxs=cw, num_idxs_reg=cw,
                    elem_size=H, single_packet=True, queue_num=qn)
                c0 += cw
            qtile = pq.tile([P, NK * H], bf16, tag="qtile")
            nc.scalar.activation(qtile[:], qg[:], AF.Copy)
            return qtile

        def emit_chunk_a(q):
            """Phase A for dst chunk q: K|V rows into kvt{q}."""
            TB = 8
            n0 = BOUNDS[q]
            while n0 < BOUNDS[q + 1]:
                tb = min(TB, (BOUNDS[q + 1] - n0) // P)
                xtb = pa.tile([P, TB, DC, P], bf16, tag="xtb")
                for c in range(DC):
                    nc.sync.dma_start(
                        xtb[:, :tb, c, :],
                        xt[c * P:(c + 1) * P, n0:n0 + tb * P].rearrange(
                            "p (t n) -> p t n", n=P))
                ps = pa_ps.tile([P, TB * H2], f32, tag="psA")
                psv = ps[:].rearrange("p (t e) -> p t e", e=H2)
                for t in range(tb):
                    for c in range(DC):
                        nc.tensor.matmul(
                            psv[:, t, :], lhsT=xtb[:, t, c, :],
                            rhs=w_sb[:, c, 0:H2],
                            start=(c == 0), stop=(c == DC - 1))
                st = pa_st.tile([P, TB * H2], bf16, tag="stA")
                nc.scalar.activation(st[:, :tb * H2], ps[:, :tb * H2], AF.Copy)
                nr0 = n0 - BOUNDS[q]
                nc.sync.dma_start(
                    kvts[q][nr0:nr0 + tb * P, :].rearrange(
                        "(t p) e -> p t e", p=P),
                    st[:, :tb * H2].rearrange("p (t e) -> p t e", e=H2))
                n0 += tb * P

        # pass-q whole-pass idx/mask prefetch tiles
        def prefetch_pass(q):
            qw_lo = kvw_cols[q][0]
            qw_hi = (kvw_cols[q + 1][0] if q < 3 else meta["KVIW"])
            iw = qw_hi - qw_lo
            it = pidx.tile([P, 4608], i16, tag="pidx")
            assert iw <= 4608, iw
            nc.sync.dma_start(it[:, :iw], kvidx[:, qw_lo:qw_hi])
            mt = pidx.tile([P, 576], f32, tag="pmsk")
            assert totw[q] <= 576, totw[q]
            nc.sync.dma_start(mt[:, :totw[q]], gmask[:, gm_off[q]:gm_off[q] + totw[q]])
            return it, mt, qw_lo

        # combine idx tiles (pass 3, resident)
        c01_sb = cpool.tile([P, meta["C01W"]], i16)
        nc.sync.dma_start(c01_sb[:], c01idx[:])
        c2_sb = cpool.tile([P, meta["C2W"]], i16)
        nc.sync.dma_start(c2_sb[:], c2idx[:])
        qgidx_sb = cpool.tile([P, 4 * meta["QGW"]], i16)
        nc.sync.dma_start(qgidx_sb[:], qgidx[:])

        # ---- startup: chunk 0 KV, Q table, pass-0 idx prefetch -------------
        emit_chunk_a(0)
        emit_qphase()
        it, mt, qw_lo = prefetch_pass(0)
        qtile = None

        # parts tables are NOT zero-initialized: the fused combine gathers
        # full 512B rows but the DVE only reads cols 0:H+1, which every
        # pass writes for every node (d >= 1 padding).  Uninitialized tail
        # bytes are moved by DMA but never consumed.

        qcycle = [0]
        for q in range(4):
            for bi, (k0, G, d, col0) in enumerate(sched[q]):
                W = G * d
                iw0 = kvw_cols[q][bi] - qw_lo
                iw = (P * W) // 16
                # SWDGE ring limit: at most 1024 idxs per gather call.
                # Cycle the 4 SWDGE queues so the 4 Q7 pairs desc-gen
                # concurrently (measured ~2x gather throughput).
                def gather_sub(out_v, in_t, idx_t, idx0, n_idx):
                    c0 = 0
                    while c0 < n_idx:
                        cw = min(1024, n_idx - c0)
                        qn = qcycle[0]
                        qcycle[0] = (qn + 1) % NQ
                        nc.gpsimd.dma_gather(
                            out_ap=out_v[:, c0 // P:(c0 + cw) // P, :],
                            in_ap=in_t[:],
                            idxs_ap=idx_t[:, idx0 + c0 // 16:idx0 + (c0 + cw) // 16],
                            num_idxs=cw, num_idxs_reg=cw,
                            elem_size=H2, single_packet=True, queue_num=qn)
                        c0 += cw

                kvg = pg.tile([P, W_CAP * H2], bf16, tag="kvg")
                kvgv = kvg[:, :W * H2].rearrange("p (w e) -> p w e", e=H2)
                gather_sub(kvgv, kvts[q], it, iw0, P * W)

                if q == 3:
                    # fused combine: gather partial rows for this batch
                    big01 = pcmb.tile([P, 2 * CG_CAP * H2], f32, tag="c01")
                    gather_sub(
                        big01[:, :2 * G * H2].rearrange("p (w e) -> p w e", e=H2),
                        parts01, c01_sb, meta["c01_cols"][bi], 2 * G * P)
                    big2 = pcmb.tile([P, CG_CAP * H2], f32, tag="c2")
                    gather_sub(
                        big2[:, :G * H2].rearrange("p (w e) -> p w e", e=H2),
                        parts2, c2_sb, meta["c2_cols"][bi], G * P)

                # interleave next-chunk/next-pass emission early in the pass
                if bi == 0 and q == 0:
                    qtile = emit_qgather(0)
                if bi == 1 and q < 3:
                    emit_chunk_a(q + 1)
                if bi == 2 and q < 3:
                    qtile_next = emit_qgather(q + 1)
                    pf_next = prefetch_pass(q + 1)

                # ---- DVE chain ----
                kv4 = kvg[:, :W * H2].rearrange("p (g j e) -> p g j e", g=G, e=H2)
                qb4 = qtile[:, k0 * H:(k0 + G) * H] \
                    .rearrange("p (g h) -> p g h", h=H) \
                    .unsqueeze(2).to_broadcast([P, G, d, H])
                qk = pbs.tile([P, W_CAP * H], bf16, tag="qk")
                qk4 = qk[:, :W * H].rearrange("p (g j h) -> p g j h", g=G, h=H)
                nc.vector.tensor_tensor(
                    out=qk4, in0=kv4[:, :, :, 0:H], in1=qb4, op=OP.mult)
                s_t = pbs.tile([P, W_CAP], f32, tag="s")
                nc.vector.tensor_reduce(
                    out=s_t[:, :W], in_=qk4, axis=AX.X, op=OP.add)
                sm = pbs.tile([P, W_CAP], f32, tag="sm")
                nc.vector.tensor_tensor(
                    out=sm[:, :W], in0=s_t[:, :W], in1=mt[:, col0:col0 + W],
                    op=OP.add)
                e_t = pbs.tile([P, W_CAP], bf16, tag="e")
                nc.scalar.activation(e_t[:, :W], sm[:, :W], AF.Exp, scale=dk_scale)
                numden = pbs.tile([P, G_CAP * (H + 1)], f32, tag="nd")
                ndv = numden[:, :G * (H + 1)].rearrange("p (g x) -> p g x", x=H + 1)
                e3 = e_t[:, :W].rearrange("p (g j) -> p g j", j=d)
                nc.vector.tensor_reduce(
                    out=ndv[:, :, H], in_=e3, axis=AX.X, op=OP.add)
                e4 = e3.unsqueeze(3).to_broadcast([P, G, d, H])
                nc.vector.tensor_tensor(
                    out=qk4, in0=kv4[:, :, :, H:H2], in1=e4, op=OP.mult)
                wv_v = qk[:, :W * H].rearrange("p (g j h) -> p g h j", g=G, h=H)
                nc.vector.tensor_reduce(
                    out=ndv[:, :, 0:H], in_=wv_v, axis=AX.X, op=OP.add)

                if q < 3:
                    r0 = k0 * P
                    tgt = parts01 if q < 2 else parts2
                    roff = q * NLOC if q < 2 else 0
                    nc.sync.dma_start(
                        tgt[roff + r0:roff + r0 + G * P, 0:H + 1].rearrange(
                            "(g p) x -> p g x", p=P),
                        ndv[:])
                else:
                    # fused combine: acc = ndv + parts0 + parts1 + parts2
                    red01 = pbs.tile([P, CG_CAP * (H + 1)], f32, tag="red01")
                    rv = red01[:, :G * (H + 1)].rearrange("p (g x) -> p g x", x=H + 1)
                    b01v = big01[:, :2 * G * H2].rearrange(
                        "p (i g e) -> p g e i", i=2, g=G, e=H2)[:, :, 0:H + 1, :]
                    nc.vector.tensor_reduce(
                        out=rv, in_=b01v, axis=AX.X, op=OP.add)
                    acc = pbs.tile([P, CG_CAP * (H + 1)], f32, tag="acc")
                    av = acc[:, :G * (H + 1)].rearrange("p (g x) -> p g x", x=H + 1)
                    nc.vector.tensor_tensor(out=av, in0=ndv, in1=rv, op=OP.add)
                    b2v = big2[:, :G * H2].rearrange(
                        "p (g e) -> p g e", e=H2)[:, :, 0:H + 1]
                    nc.vector.tensor_tensor(out=av, in0=av, in1=b2v, op=OP.add)
                    dcl = pbs.tile([P, CG_CAP], f32, tag="dcl")
                    nc.vector.tensor_scalar_max(
                        out=dcl[:, :G], in0=av[:, :, H], scalar1=1e-38)
                    rcp = pbs.tile([P, CG_CAP], f32, tag="rcp")
                    nc.vector.reciprocal(rcp[:, :G], dcl[:, :G])
                    ob = pbs.tile([P, CG_CAP * H], f32, tag="ob")
                    nc.vector.tensor_tensor(
                        out=ob[:, :G * H].rearrange("p (g h) -> p g h", h=H),
                        in0=av[:, :, 0:H],
                        in1=rcp[:, :G].unsqueeze(2).to_broadcast([P, G, H]),
                        op=OP.mult)
                    nc.sync.dma_start(
                        out[k0 * P:(k0 + G) * P, :].rearrange(
                            "(g p) h -> p g h", p=P),
                        ob[:, :G * H])

            if q < 3:
                qtile = qtile_next
                it, mt, qw_lo = pf_next

    nc.finalize()
    return nc


_CACHE = {}


def _get_program(cfg, meta):
    key = (cfg["N"], cfg["D"], cfg["H"],
           str(meta["sched"]), meta["KVIW"], meta["TOTW"])
    if key not in _CACHE:
        _CACHE[key] = _build_program(cfg, meta)
    return _CACHE[key]


def run(X, Wq, Wk, Wv, edge_index, trace=False, tmpdir=None):
    from concourse.bass_utils import run_bass_kernel_spmd

    X = np.asarray(X)
    N, D = X.shape
    H = np.asarray(Wq).shape[1]
    cfg = _cfg_from_shapes(N, D, H)
    meta, in_maps, post = _prep(cfg, X, Wq, Wk, Wv, edge_index)
    nc = _get_program(cfg, meta)
    res = run_bass_kernel_spmd(
        nc, in_maps, list(range(NC)), trace=trace, tmpdir=tmpdir)

    NLOC, NDUM = cfg["NLOC"], post["NDUM"]
    order = post["order"]
    pq_order3 = post["pq_order3"]
    out_pos = np.empty((cfg["NPOS"], H), np.float32)
    kk = np.arange(NLOC)
    for c in range(NC):
        gpos = ((kk // P) * NC + c) * P + kk % P
        # device rows are in pass-3 order: row r holds node pq_order3[c, r]
        out_pos[gpos[pq_order3[c]]] = res.results[c]["out"]
    out_full = np.empty((N, H), np.float32)
    out_full[order] = out_pos[NDUM:]
    return out_full, res


def kernel(X, Wq, Wk, Wv, edge_index):
    out, _ = run(X, Wq, Wk, Wv, edge_index, trace=False)
    return out
